# revision 5
# baseline (speedup 1.0000x reference)
"""PhaseAttention TRN2 kernel: complex linear projections + sliding-window
(256) causal attention + complex output projection, sharded over 8 cores as
(batch=2) x (4-head groups=4). Each core computes q/k/v for its 4 heads via a
stacked real contraction (K=2048), windowed attention in scores-transposed
orientation (no PE transposes), and a partial Wout matmul; the host sums the
4 partials per batch."""

import numpy as np

B, L, D = 2, 4096, 1024
NH, HD, WIN = 16, 64, 256
HPC = 4            # heads per core
SCALE = HD ** -0.5
QB = 256           # query block
NQB = L // QB      # 16
LCH = 512          # pass-1 L chunk
NLCH = L // LCH    # 8

_CACHE = {}


def _build_program():
    from concourse import bacc, tile, mybir

    F32R = mybir.dt.float32r
    F32 = mybir.dt.float32
    EXP = mybir.ActivationFunctionType.Exp

    nc = bacc.Bacc(None, target_bir_lowering=False, debug=True)
    zt_h = nc.declare_dram_parameter("zt", [2 * D, L], F32R, isOutput=False)
    wq_h = nc.declare_dram_parameter("wq", [2 * D, 512], F32R, isOutput=False)
    wk_h = nc.declare_dram_parameter("wk", [2 * D, 512], F32R, isOutput=False)
    wv_h = nc.declare_dram_parameter("wv", [2 * D, 512], F32R, isOutput=False)
    wo_h = nc.declare_dram_parameter("wo", [512, 2 * D], F32R, isOutput=False)
    mk_h = nc.declare_dram_parameter("mk", [128, 1024], F32R, isOutput=False)
    out_h = nc.declare_dram_parameter("out", [L, 2 * D], F32, isOutput=True)

    qt_s = nc.dram_tensor("qt_s", [512, L], F32R)
    kt_s = nc.dram_tensor("kt_s", [512, L], F32R)
    v_s = nc.dram_tensor("v_s", [L, 512], F32R)

    zt_r = zt_h.ap().rearrange("(k p) l -> p k l", p=128)      # [128,16,L]
    qt_r = qt_s.ap().rearrange("(h p) l -> p h l", p=128)      # [128,4,L]
    kt_r = kt_s.ap().rearrange("(h p) l -> p h l", p=128)
    v_r = v_s.ap().rearrange("(n p) d -> p n d", p=128)        # [128,32,512]
    wo_r = wo_h.ap().rearrange("(h p) n -> p h n", p=128)      # [128,4,2048]

    with tile.TileContext(nc) as tc:
        # ---- pass 1: qT/kT [512,L] and v [L,512] projections ----
        with (
            tc.tile_pool(name="wst", bufs=1) as wst,
            tc.tile_pool(name="zp", bufs=2) as zp,
            tc.tile_pool(name="st1", bufs=4) as st1,
            tc.tile_pool(name="pp1", bufs=2, space="PSUM") as pp1,
        ):
            w_tiles = {}
            for nm, h in (("q", wq_h), ("k", wk_h), ("v", wv_h)):
                t = wst.tile([128, 16, 512], F32R, name=f"w_{nm}")
                nc.sync.dma_start(
                    out=t[:], in_=h.ap().rearrange("(k p) n -> p k n", p=128)
                )
                w_tiles[nm] = t

            for lc in range(NLCH):
                zt = zp.tile([128, 16, LCH], F32R)
                nc.sync.dma_start(
                    out=zt[:], in_=zt_r[:, :, lc * LCH:(lc + 1) * LCH]
                )
                for nm, scr in (("q", qt_r), ("k", kt_r)):
                    wt = w_tiles[nm]
                    for m in range(4):
                        p = pp1.tile([128, LCH], F32)
                        for kk in range(16):
                            nc.tensor.matmul(
                                p[:],
                                wt[:, kk, m * 128:(m + 1) * 128],
                                zt[:, kk, :],
                                start=(kk == 0),
                                stop=(kk == 15),
                            )
                        s = st1.tile([128, LCH], F32R)
                        nc.scalar.copy(s[:], p[:])
                        nc.sync.dma_start(
                            out=scr[:, m, lc * LCH:(lc + 1) * LCH], in_=s[:]
                        )
                wt = w_tiles["v"]
                for lb in range(4):
                    p = pp1.tile([128, 512], F32)
                    for kk in range(16):
                        nc.tensor.matmul(
                            p[:],
                            zt[:, kk, lb * 128:(lb + 1) * 128],
                            wt[:, kk, :],
                            start=(kk == 0),
                            stop=(kk == 15),
                        )
                    s = st1.tile([128, 512], F32R)
                    nc.scalar.copy(s[:], p[:])
                    nc.sync.dma_start(out=v_r[:, lc * 4 + lb, :], in_=s[:])

        # ---- pass 2: windowed attention + Wout partial ----
        with (
            tc.tile_pool(name="cst", bufs=1) as cst,
            tc.tile_pool(name="qkv2", bufs=2) as qkv2,
            tc.tile_pool(name="ep", bufs=8) as ep,
            tc.tile_pool(name="ap2", bufs=8) as ap2,
            tc.tile_pool(name="nrm", bufs=4) as nrm,
            tc.tile_pool(name="ost", bufs=2) as ost,
            tc.tile_pool(name="psc", bufs=2, space="PSUM") as psc,
            tc.tile_pool(name="prs", bufs=2, space="PSUM") as prs,
            tc.tile_pool(name="pov", bufs=2, space="PSUM") as pov,
            tc.tile_pool(name="pwo", bufs=2, space="PSUM") as pwo,
        ):
            mk_t = cst.tile([128, 1024], F32R)
            nc.sync.dma_start(out=mk_t[:], in_=mk_h.ap())
            wo_t = cst.tile([128, 4, 2048], F32R)
            nc.sync.dma_start(out=wo_t[:], in_=wo_r)
            ones_f = cst.tile([128, 1], F32)
            nc.vector.memset(ones_f[:], 1.0)
            ones = cst.tile([128, 1], F32R)
            nc.scalar.copy(ones[:], ones_f[:])

            for qb in range(NQB):
                qs = qb * QB
                r_lo = 2 if qb == 0 else 0
                qt_t = qkv2.tile([128, 4, QB], F32R, tag="qt")
                nc.sync.dma_start(out=qt_t[:], in_=qt_r[:, :, qs:qs + QB])
                kt_t = qkv2.tile([128, 4, 512], F32R, tag="kt")
                v_t = qkv2.tile([128, 4, 512], F32R, tag="vt")
                if qb == 0:
                    nc.sync.dma_start(
                        out=kt_t[:, :, 256:], in_=kt_r[:, :, 0:256]
                    )
                    nc.sync.dma_start(out=v_t[:, 2:4, :], in_=v_r[:, 0:2, :])
                else:
                    w0 = qs - 256
                    nc.sync.dma_start(
                        out=kt_t[:], in_=kt_r[:, :, w0:w0 + 512]
                    )
                    n0 = w0 // 128
                    nc.sync.dma_start(out=v_t[:], in_=v_r[:, n0:n0 + 4, :])

                att_tiles = []
                for h in range(HPC):
                    e_tiles = {}
                    for r in range(r_lo, 4):
                        sc_p = psc.tile([128, QB], F32)
                        nc.tensor.matmul(
                            sc_p[:],
                            kt_t[:, h, r * 128:(r + 1) * 128],
                            qt_t[:, h, :],
                            start=True,
                            stop=True,
                        )
                        e_t = ep.tile([128, QB], F32R)
                        nc.scalar.activation(e_t[:], sc_p[:], EXP, scale=SCALE)
                        nc.vector.tensor_mul(
                            e_t[:], e_t[:], mk_t[:, r * 256:(r + 1) * 256]
                        )
                        e_tiles[r] = e_t
                    rs_p = prs.tile([1, QB], F32)
                    o_p = pov.tile([128, QB], F32)
                    for i, r in enumerate(range(r_lo, 4)):
                        st_f = (i == 0)
                        sp_f = (r == 3)
                        nc.tensor.matmul(
                            rs_p[:], ones[:], e_tiles[r][:],
                            start=st_f, stop=sp_f,
                        )
                        nc.tensor.matmul(
                            o_p[:],
                            v_t[:, r, h * 128:(h + 1) * 128],
                            e_tiles[r][:],
                            start=st_f, stop=sp_f,
                        )
                    rc_t = nrm.tile([1, QB], F32, tag="rc")
                    nc.vector.reciprocal(rc_t[:], rs_p[:])
                    rb_t = nrm.tile([128, QB], F32, tag="rb")
                    nc.gpsimd.partition_broadcast(rb_t[:], rc_t[:])
                    a_t = ap2.tile([128, QB], F32R)
                    nc.vector.tensor_mul(a_t[:], o_p[:], rb_t[:])
                    att_tiles.append(a_t)

                for lt in range(2):
                    o_st = ost.tile([128, 2048], F32)
                    for n in range(4):
                        wp = pwo.tile([128, 512], F32)
                        for h in range(HPC):
                            nc.tensor.matmul(
                                wp[:],
                                att_tiles[h][:, lt * 128:(lt + 1) * 128],
                                wo_t[:, h, n * 512:(n + 1) * 512],
                                start=(h == 0),
                                stop=(h == 3),
                            )
                        nc.scalar.copy(o_st[:, n * 512:(n + 1) * 512], wp[:])
                    ls = qs + lt * 128
                    nc.sync.dma_start(
                        out=out_h.ap()[ls:ls + 128, :], in_=o_st[:]
                    )

    nc.finalize()
    return nc


def _masks():
    kk = np.arange(128)[:, None]
    qq = np.arange(256)[None, :]
    m = np.empty((128, 1024), np.float32)
    m[:, 0:256] = kk >= qq + 1
    m[:, 256:512] = kk >= qq - 127
    m[:, 512:768] = kk <= qq
    m[:, 768:1024] = kk <= qq - 128
    return m


def _core_inputs(z, Wq, Wk, Wv, Wout):
    masks = _masks()
    zt = {}
    for b in range(B):
        zt[b] = np.ascontiguousarray(
            np.concatenate([z[b, :, :, 0].T, z[b, :, :, 1].T], axis=0)
        ).astype(np.float32)

    def proj_stack(W, qtr):
        Wr, Wi = W[:, :, 0], W[:, :, 1]
        st = np.empty((2 * D, 512), np.float32)
        for j in range(HPC):
            h = HPC * qtr + j
            ch = slice(HD * h, HD * (h + 1))
            c0 = 128 * j
            st[:D, c0:c0 + 64] = Wr[:, ch]
            st[D:, c0:c0 + 64] = -Wi[:, ch]
            st[:D, c0 + 64:c0 + 128] = Wi[:, ch]
            st[D:, c0 + 64:c0 + 128] = Wr[:, ch]
        return st

    def out_stack(W, qtr):
        Wr, Wi = W[:, :, 0], W[:, :, 1]
        st = np.empty((512, 2 * D), np.float32)
        for j in range(HPC):
            h = HPC * qtr + j
            ch = slice(HD * h, HD * (h + 1))
            r0 = 128 * j
            st[r0:r0 + 64, :D] = Wr[ch, :]
            st[r0:r0 + 64, D:] = Wi[ch, :]
            st[r0 + 64:r0 + 128, :D] = -Wi[ch, :]
            st[r0 + 64:r0 + 128, D:] = Wr[ch, :]
        return st

    in_maps = []
    for c in range(8):
        b, qtr = divmod(c, 4)
        in_maps.append({
            "zt": zt[b],
            "wq": proj_stack(Wq, qtr),
            "wk": proj_stack(Wk, qtr),
            "wv": proj_stack(Wv, qtr),
            "wo": out_stack(Wout, qtr),
            "mk": masks,
        })
    return in_maps


def kernel(z, Wq, Wk, Wv, Wout, _trace=False):
    from concourse.bass_utils import run_bass_kernel_spmd

    z = np.asarray(z, np.float32)
    Wq, Wk, Wv, Wout = (np.asarray(w, np.float32) for w in (Wq, Wk, Wv, Wout))

    if "nc" not in _CACHE:
        _CACHE["nc"] = _build_program()
    nc = _CACHE["nc"]

    in_maps = _core_inputs(z, Wq, Wk, Wv, Wout)
    res = run_bass_kernel_spmd(nc, in_maps, list(range(8)), trace=_trace)
    _CACHE["last_exec_time_ns"] = res.exec_time_ns

    out = np.zeros((B, L, D, 2), np.float32)
    for c in range(8):
        b = c // 4
        p = res.results[c]["out"]
        out[b, :, :, 0] += p[:, :D]
        out[b, :, :, 1] += p[:, D:]
    return out
